# revision 1
# baseline (speedup 1.0000x reference)
"""Trainium2 Bass kernel for batched cross-attention with gaussian guide mask.

Reference computation (per batch b):
  Q   = query @ Wq.T                      # [Tq, A]
  att = (Q @ K.T / sqrt(A)) * guide       # guide[n] = exp(-(step-(n+1)/N)^2/TEMP)
  att = where(mask, -inf, att)
  out = softmax(att, axis=-1) @ V         # [Tq, E]

Sharding: data-parallel over batch. Core b handles batch b (B == 8 == n_cores).

Layout strategy (v2, transposed attention):
  The attention matrix is computed TRANSPOSED: attT[n, t] = sum_a K[n,a]Q[t,a],
  via matmul(lhsT=K^T tile [a,n], rhs=Q^T [a,t]). The softmax scores s^T[n, t]
  are then directly the stationary operand of the AV matmul
  (out[t,e] = sum_n s^T[n,t]^T V[n,e]) - no PE transposes needed at all.
  Because the guide factor varies along n (the PARTITION dim of attT), the
  guide * 1/sqrt(A) scale folds into the exp as a per-partition activation
  scale: s = exp(guide_n * attT). Masking applies AFTER exp (predicated zero
  with mask^T tiles). The softmax row-sum (a partition-dim reduction here) is
  a DVE tree-add of the 16 n-tiles of s^T followed by one 1-column matmul
  against a ones vector per 128-row output tile.
Host does layout-only prep (transpose/cast/f16/u8, chunk-contiguous rows);
output is stored f16 and widened to f32 on the host.
"""

import math

import numpy as np

import concourse.bass as bass
import concourse.mybir as mybir
import concourse.tile as tile
from concourse import bacc
from concourse.bass import ts
from concourse.bass_utils import run_bass_kernel_spmd

B, TQ, N = 8, 1024, 2048
L, A, E = 1024, 128, 512
TEMP = 0.08
P = 128
LT = L // P    # 8 l-tiles (contraction tiles of the Q projection)
NT = N // P    # 16 n-tiles
SB = 512       # t columns per superblock
NSB = TQ // SB  # 2 superblocks
TPS = SB // P  # 4 t-tiles per superblock

F32 = mybir.dt.float32
F16 = mybir.dt.float16
U8 = mybir.dt.uint8
H64 = P // 2


def build_nc():
    nc = bacc.Bacc("TRN2", target_bir_lowering=False, debug=False, enable_asserts=False, num_devices=B)

    qtaA = nc.dram_tensor("qtaA", [P, LT * P], F16, kind="ExternalInput").ap()
    qtaB = nc.dram_tensor("qtaB", [P, LT * (SB - P)], F16, kind="ExternalInput").ap()
    qtb = nc.dram_tensor("qtb", [P, LT * SB], F16, kind="ExternalInput").ap()
    kT = nc.dram_tensor("kT", [A, N], F16, kind="ExternalInput").ap()
    v = nc.dram_tensor("v", [P, NT * E], F16, kind="ExternalInput").ap()
    wqd = nc.dram_tensor("wqd", [P, LT * A], F16, kind="ExternalInput").ap()
    stp = nc.dram_tensor("stp", [1, 1], F32, kind="ExternalInput").ap()
    mska = nc.dram_tensor("mska", [P, NT * SB], U8, kind="ExternalInput").ap()
    mskb = nc.dram_tensor("mskb", [P, NT * SB], U8, kind="ExternalInput").ap()
    out = nc.dram_tensor("out", [P, NSB * TPS * E], F16, kind="ExternalOutput").ap()

    with tile.TileContext(nc) as tc:
        with (
            tc.tile_pool(name="const", bufs=1) as const,
            tc.tile_pool(name="setup", bufs=1) as setup,
            tc.tile_pool(name="stp_", bufs=2) as stpool,
            tc.tile_pool(name="mpool", bufs=2) as mpool,
            tc.tile_pool(name="opool", bufs=2) as opool,
            tc.tile_pool(name="tree", bufs=2) as tree,
            tc.tile_pool(name="rsp", bufs=2) as rsp,
            tc.tile_pool(name="small", bufs=8) as small,
            tc.tile_pool(name="psA", bufs=5, space="PSUM") as psA,
            tc.tile_pool(name="psO", bufs=2, space="PSUM") as psO,
            tc.tile_pool(name="psR", bufs=1, space="PSUM") as psR,
        ):
            # ---- one-time setup: constants + all input DMA triggers ----
            # step arrives as a single packet into one partition and is
            # broadcast across partitions with a 1-contraction matmul (keeps
            # the 128-packet broadcast off the DMA queues).
            step1 = const.tile([1, 1], F32)
            nc.sync.dma_start(out=step1, in_=stp)
            ones1 = const.tile([1, P], F32)
            nc.vector.memset(ones1, 1.0)
            # pos_col[p, j] = (j*128 + p) + 1  -> n+1 for n-tile j, lane p
            posc = const.tile([P, NT], F32)
            nc.gpsimd.iota(
                posc,
                pattern=[[P, NT]],
                base=1,
                channel_multiplier=1,
                allow_small_or_imprecise_dtypes=True,
            )

            # sync + scalar HW-DGE queues carry the startup-critical loads
            # (wq, query, K); the software-DGE queue (fast once its ~5us
            # pipeline latency passes) carries masks + V.
            wq_sb = setup.tile([P, LT, A], F16)
            qtin0a = setup.tile([P, LT, P], F16)
            qtin0b = setup.tile([P, LT, SB - P], F16)
            qtin1 = setup.tile([P, LT, SB], F16)
            kt_sb = const.tile([P, N], F16)
            msk1 = mpool.tile([P, NT, SB], U8, name="msk")
            m1f = msk1.rearrange("p n t -> p (n t)")
            wqf = wq_sb.rearrange("p l a -> p (l a)")
            v_sb = const.tile([P, NT, E], F16)
            msk0 = mpool.tile([P, NT, SB], U8, name="msk")
            m0f = msk0.rearrange("p n t -> p (n t)")
            # scalar gets exactly 4 input triggers (its DMA-sem pool depth) so
            # the engine never stalls on a trigger wrap before the exp stream.
            q0af = qtin0a.rearrange("p l t -> p (l t)")
            q0bf = qtin0b.rearrange("p l t -> p (l t)")
            q1f = qtin1.rearrange("p l t -> p (l t)")
            nc.sync.dma_start(out=wqf[: H64, :], in_=wqd[: H64, :])
            nc.scalar.dma_start(out=wqf[H64:, :], in_=wqd[H64:, :])
            for eng, h in ((nc.sync, 0), (nc.scalar, 1)):
                hs = slice(h * H64, (h + 1) * H64)
                eng.dma_start(out=q0af[hs, :], in_=qtaA[hs, :])
                eng.dma_start(out=q0bf[hs, :], in_=qtaB[hs, :])
            nc.sync.dma_start(out=kt_sb[: H64, :], in_=kT[: H64, :])
            nc.scalar.dma_start(out=kt_sb[H64:, :], in_=kT[H64:, :])
            # scalar takes one more input (trigger 5 wrap-stalls the engine
            # only until transfer 1 lands — well before the exp stream)
            nc.scalar.dma_start(out=m0f[:, NT * SB // 2 :], in_=mska[:, NT * SB // 2 :])
            nc.sync.dma_start(out=q1f[: H64, :], in_=qtb[: H64, :])
            # software-DGE: first V pair, mask sb0 head, qtb half, even V
            # pairs, mskb half; V pairs alternate SW / sync in AV-contraction
            # order so the first AV chain cascades just-in-time
            nc.gpsimd.dma_start(
                out=v_sb[:, 0:2, :].rearrange("p n e -> p (n e)"), in_=v[:, : 2 * E]
            )
            nc.gpsimd.dma_start(out=m0f[:, : NT * SB // 2], in_=mska[:, : NT * SB // 2])
            nc.gpsimd.dma_start(out=q1f[H64:, :], in_=qtb[H64:, :])
            for g in range(1, 8):
                eng = nc.sync if g % 2 == 1 else nc.gpsimd
                eng.dma_start(
                    out=v_sb[:, ts(g, 2), :].rearrange("p n e -> p (n e)"),
                    in_=v[:, ts(g, 2 * E)],
                )
            nc.sync.dma_start(out=m1f[: H64, :], in_=mskb[: H64, :])
            nc.gpsimd.dma_start(out=m1f[H64:, :], in_=mskb[H64:, :])

            ones_sb = const.tile([P, 1], F16)
            nc.vector.memset(ones_sb, 1.0)
            zero_sb = const.tile([P, SB], F16)
            nc.vector.memset(zero_sb, 0.0)

            # guide chain (cheap: [128, 16] tiles)
            ps_step = psR.tile([P, 1], F32, tag="rs", name="ps_step")
            nc.tensor.matmul(ps_step, ones1, step1, start=True, stop=True)
            nstep = const.tile([P, 1], F32)
            nc.vector.tensor_scalar_mul(nstep, ps_step, -1.0)
            gbias = const.tile([P, 1], F32)
            nc.vector.memset(gbias, -0.5 * math.log(A))
            zsq = const.tile([P, NT], F32)
            nc.scalar.activation(
                out=zsq, in_=posc, func=mybir.ActivationFunctionType.Square,
                bias=nstep, scale=1.0 / N,
            )
            # guide[p, j] = exp(-(step-(n+1)/N)^2/TEMP) / sqrt(A), n = j*128+p
            guide = const.tile([P, NT], F32)
            nc.scalar.activation(
                out=guide, in_=zsq, func=mybir.ActivationFunctionType.Exp,
                scale=-1.0 / TEMP, bias=gbias,
            )

            qt = const.tile([P, TQ], F16)      # projected Q^T [a, t]
            st = {}                            # per-superblock score tiles s^T
            obs = {}                           # per-superblock output rows
            rsum = {}                          # per-superblock row-sum partials

            def proj_chunk(qin, col0, width, copy_eng):
                # qt[:, col0:col0+width] = (Wq @ query^T)[a, t], 8 l-tile steps
                pq = psA.tile([P, width], F32, tag="att", name="pq")
                for lt in range(LT):
                    nc.tensor.matmul(
                        pq, wq_sb[:, lt, :], qin[:, lt, :],
                        start=(lt == 0), stop=(lt == LT - 1),
                    )
                if copy_eng is nc.scalar:
                    copy_eng.copy(qt[:, col0 : col0 + width], pq)
                else:
                    copy_eng.tensor_copy(qt[:, col0 : col0 + width], pq)

            def att_tile(sb, nt, pool, do_pred=True):
                # attT[n, t] (one n-tile), then s^T = exp(guide_n * attT), then
                # masked lanes -> 0
                ps = pool.tile([P, SB], F32, tag="att", name="attT")
                nc.tensor.matmul(ps, kt_sb[:, ts(nt, P)], qt[:, ts(sb, SB)], start=True, stop=True)
                nc.scalar.activation(
                    out=st[sb][:, nt, :], in_=ps,
                    func=mybir.ActivationFunctionType.Exp,
                    scale=guide[:, nt : nt + 1],
                )
                if do_pred:
                    pred_tile(sb, nt)

            def pred_tile(sb, nt):
                msk = msk0 if sb == 0 else msk1
                nc.vector.copy_predicated(
                    out=st[sb][:, nt, :], mask=msk[:, nt, :], data=zero_sb,
                )

            def rowsum_tree(sb):
                # partial[n', t] = sum over the 16 n-tiles of s^T (DVE tree)
                s = st[sb]
                t8 = tree.tile([P, 8, SB], F16, name="t8")
                nc.vector.tensor_tensor(
                    out=t8, in0=s[:, 0:8, :], in1=s[:, 8:16, :], op=mybir.AluOpType.add
                )
                t4 = tree.tile([P, 4, SB], F16, name="t4")
                nc.vector.tensor_tensor(
                    out=t4, in0=t8[:, 0:4, :], in1=t8[:, 4:8, :], op=mybir.AluOpType.add
                )
                t2 = tree.tile([P, 2, SB], F16, name="t2")
                nc.vector.tensor_tensor(
                    out=t2, in0=t4[:, 0:2, :], in1=t4[:, 2:4, :], op=mybir.AluOpType.add
                )
                rsum[sb] = rsp.tile([P, SB], F16, name="rsum")
                nc.vector.tensor_tensor(
                    out=rsum[sb], in0=t2[:, 0, :], in1=t2[:, 1, :], op=mybir.AluOpType.add
                )

            def av_tile(sb, tt, extras=()):
                # out[t, e] = sum_n s[t, n] V[n, e], contraction over 16 n-tiles.
                # `extras` are thunks (attT matmuls of the next superblock)
                # woven between chain segments to fill PE slots.
                # row-sum first (independent of the chain): rs[t] = sum_n'
                # partial[n', t] via 1-col matmul, so the final norm only
                # waits on the chain itself
                rs = psR.tile([P, 1], F32, tag="rs", name="rs")
                nc.tensor.matmul(rs, rsum[sb][:, ts(tt, P)], ones_sb, start=True, stop=True)
                rc = small.tile([P, 1], F32, name="rc")
                nc.vector.reciprocal(rc, rs)
                ot = psO.tile([P, E], F32, tag="pso", name="ot")
                extras = list(extras)
                for nt in range(NT):
                    nc.tensor.matmul(
                        ot, st[sb][:, nt, ts(tt, P)], v_sb[:, nt, :],
                        start=(nt == 0), stop=(nt == NT - 1),
                    )
                    if nt % 2 == 1 and extras:
                        extras.pop(0)()
                nc.vector.tensor_scalar_mul(obs[sb][:, tt, :], ot, rc)

            def store(sb):
                obf = obs[sb].rearrange("p c e -> p (c e)")
                for eng, h in ((nc.sync, 0), (nc.scalar, 1)):
                    hs = slice(h * H64, (h + 1) * H64)
                    eng.dma_start(out=out[hs, ts(sb, TPS * E)], in_=obf[hs, :])

            # ---- main flow ----
            # attT(sb1) matmuls are woven inside the AV(sb0) chains (dedicated
            # psB banks) so their exps run in AV(sb0)'s shadow without ever
            # blocking the chain; sb1 predicates slot in after each norm so
            # the in-order DVE queue never blocks the psO rotation.
            proj_chunk(qtin0a, 0, P, nc.vector)
            proj_chunk(qtin0b, P, SB - P, nc.vector)
            st[0] = stpool.tile([P, NT, SB], F16, name="st")
            for nt in range(NT):
                att_tile(0, nt, psA)
            rowsum_tree(0)
            obs[0] = opool.tile([P, TPS, E], F16, name="ob")
            st[1] = stpool.tile([P, NT, SB], F16, name="st")
            # proj1 after av(0,0) so a late qtb can never head-block the
            # first AV chain; attT(sb1) weaves through blocks tt1/tt2
            av_tile(0, 0)
            proj_chunk(qtin1, SB, SB, nc.scalar)
            weave = [(), range(0, 8), range(8, NT), ()]
            pred_batches = [(0, 0), (0, 8), (8, 16), (16, 16)]
            for tt in range(1, TPS):
                extras = [
                    (lambda nt=nt: att_tile(1, nt, psA, do_pred=False))
                    for nt in weave[tt]
                ]
                av_tile(0, tt, extras)
                for nt in range(*pred_batches[tt]):
                    pred_tile(1, nt)
            rowsum_tree(1)
            store(0)
            obs[1] = opool.tile([P, TPS, E], F16, name="ob")
            for tt in range(TPS):
                av_tile(1, tt)
            store(1)

    nc.compile()
    return nc


def make_in_maps(query, K, V, Wq, step, mask):
    query = np.asarray(query, dtype=np.float32)
    K = np.asarray(K, dtype=np.float32)
    V = np.asarray(V, dtype=np.float32)
    Wq = np.asarray(Wq, dtype=np.float32)
    step = np.asarray(step, dtype=np.float32)
    mask = np.asarray(mask)
    if mask.dtype != np.uint8:
        mask = mask.astype(np.uint8)

    # wq[p, lt, a] = Wq[a, lt*128+p]
    wq_arr = np.ascontiguousarray(
        Wq.T.astype(np.float16).reshape(LT, P, A).transpose(1, 0, 2).reshape(P, LT * A)
    )
    stp = step.reshape(1, 1)
    in_maps = []
    for b in range(B):
        # qt[p, lt, t] = query[b][t, lt*128+p]; split by t halves
        qt_full = query[b].T.astype(np.float16).reshape(LT, P, TQ).transpose(1, 0, 2)
        # mskT[p, nt, t] = mask[b][t, nt*128+p]; split by t halves
        mt_full = mask[b].T.reshape(NT, P, TQ).transpose(1, 0, 2)
        in_maps.append(
            {
                "qtaA": np.ascontiguousarray(qt_full[:, :, :P]).reshape(P, LT * P),
                "qtaB": np.ascontiguousarray(qt_full[:, :, P:SB]).reshape(P, LT * (SB - P)),
                "qtb": np.ascontiguousarray(qt_full[:, :, SB:]).reshape(P, LT * SB),
                "kT": np.ascontiguousarray(K[b].T).astype(np.float16),
                "v": np.ascontiguousarray(
                    V[b].astype(np.float16).reshape(NT, P, E).transpose(1, 0, 2)
                ).reshape(P, NT * E),
                "wqd": wq_arr,
                "stp": stp,
                "mska": np.ascontiguousarray(mt_full[:, :, :SB]).reshape(P, NT * SB),
                "mskb": np.ascontiguousarray(mt_full[:, :, SB:]).reshape(P, NT * SB),
            }
        )
    return in_maps


def gather_out(res):
    outs = []
    for b in range(B):
        # out[p, sb*4*512 + tt*512 + e] -> out[b, sb*512 + tt*128 + p, e]
        o = res.results[b]["out"].reshape(P, NSB, TPS, E)
        outs.append(o.transpose(1, 2, 0, 3).reshape(TQ, E).astype(np.float32))
    return np.stack(outs, axis=0)


def kernel(query, K, V, Wq, step, mask):
    nc = build_nc()
    in_maps = make_in_maps(query, K, V, Wq, step, mask)
    res = run_bass_kernel_spmd(nc, in_maps, core_ids=list(range(B)))
    return gather_out(res)


if __name__ == "__main__":
    rng = np.random.default_rng(0)
    inputs = {
        "query": rng.standard_normal((B, TQ, L), dtype=np.float32),
        "K": rng.standard_normal((B, N, A), dtype=np.float32),
        "V": rng.standard_normal((B, N, E), dtype=np.float32),
        "Wq": rng.standard_normal((A, L), dtype=np.float32) / math.sqrt(L),
        "step": rng.random((1,), dtype=np.float32),
        "mask": rng.integers(0, 2, size=(B, TQ, N)) > 0,
    }
    out = kernel(**inputs)
    print(out.shape, out.dtype)

